# revision 9
# baseline (speedup 1.0000x reference)
"""EuclideanCodebook (VQ nearest-codebook quantization) on 8 Trainium2 NeuronCores.

Reference computation (per token x of dim 128, codebook embed [1024, 128]):
    ind = argmax_k -(||x||^2 - 2 x.e_k + ||e_k||^2)   (first index wins ties)
    quantized = embed[ind]

Since ||x||^2 is constant per token, ind = argmax_k (x.e_k - ||e_k||^2/2).

Sharding: data-parallel over the flattened token axis (65536 tokens -> 8192
per core); the [1024, 128] codebook is replicated. No collectives needed
(eval-mode forward only).

Per-core device algorithm, per 128-token chunk:
  1. PE transposes the x chunk (tokens x dim -> dim x tokens).
  2. PE matmul (fp32, exact): v = xT.T @ embT -> PSUM [128 tok, 1024 codes].
  3. DVE tensor_tensor_reduce: dist = v - esq/2 (written to SBUF) and
     maxv = row max, in one pass.
  4. DVE scalar_tensor_tensor: accum of (dist >= maxv) * iota -> argmax index.
  5. GPSIMD indirect DMA gathers embed[ind] rows; regular DMA stores them.
"""

import os
import numpy as np

import concourse.bacc as bacc
import concourse.mybir as mybir
import concourse.tile as tile
from concourse.bass import IndirectOffsetOnAxis
from concourse.bass_utils import run_bass_kernel_spmd
from concourse.masks import make_identity

P = 128           # partitions / tokens per chunk
D = 128           # embedding dim
K = 1024          # codebook size
NCORES = 8
NTOK = 16 * 4096  # full token count
TPC = NTOK // NCORES  # tokens per core

f32 = mybir.dt.float32
f32r = mybir.dt.float32r
bf16 = mybir.dt.bfloat16
fp16 = mybir.dt.float16
i32 = mybir.dt.int32

NEG_INIT = -3.0e38


def build(tpc=TPC, group=4, main_dtype="float32"):
    """Build the single-core Bass kernel (same program runs on all 8 cores)."""
    nchunk = tpc // P
    ngroup = nchunk // group
    assert ngroup * group == nchunk

    nc = bacc.Bacc("TRN2")
    x = nc.dram_tensor("x", [tpc, D], f32, kind="ExternalInput")
    embed = nc.dram_tensor("embed", [K, D], f32, kind="ExternalInput")
    quant = nc.dram_tensor("quant", [tpc, D], f32, kind="ExternalOutput")
    ind = nc.dram_tensor("ind", [tpc], i32, kind="ExternalOutput")

    with tile.TileContext(nc) as tc:
        with (
            tc.tile_pool(name="const", bufs=1) as const,
            tc.tile_pool(name="psA", bufs=2, space="PSUM") as psA,
            tc.tile_pool(name="psV", bufs=2, space="PSUM") as psV,
            tc.tile_pool(name="io", bufs=2) as io_pool,
            tc.tile_pool(name="work", bufs=3) as work,
        ):
            # ---------------- one-time setup ----------------
            identity = const.tile([P, P], f32)
            make_identity(nc, identity[:])

            # embed [1024, 128] -> SBUF as [p=code%128, c=code//128, d]
            e_sb = const.tile([P, 8, D], f32)
            nc.sync.dma_start(out=e_sb[:], in_=embed[:].rearrange("(c p) d -> p c d", p=P))

            # embT [dim, code] via 8 PE transposes
            embT = const.tile([P, K], f32)
            for c in range(8):
                tps = psA.tile([P, P], f32, tag="t128")
                nc.tensor.transpose(tps[:], e_sb[:, c, :], identity[:])
                nc.scalar.copy(embT[:, c * P:(c + 1) * P], tps[:])

            # c_row [1, 1024] = -||e_k||^2/2, from column sums of embT^2
            embT2 = const.tile([P, K], f32)
            nc.vector.tensor_mul(embT2[:], embT[:], embT[:])
            ones_col = const.tile([P, 1], f32)
            nc.vector.memset(ones_col[:], 1.0)
            c_row = const.tile([1, K], f32)
            for h in range(2):
                sl = slice(h * 512, (h + 1) * 512)
                eps = psA.tile([1, 512], f32, tag="t128")
                nc.tensor.matmul(eps[:], lhsT=ones_col[:], rhs=embT2[:, sl],
                                 start=True, stop=True)
                nc.scalar.activation(c_row[:, sl], eps[:],
                                     mybir.ActivationFunctionType.Copy,
                                     scale=-0.5)

            # exact bf16 triple split of c_row: c3[0]+c3[1]+c3[2] ~= c_row
            # (error ~1e-5, far below fp32 rounding of the distances)
            c3 = const.tile([3, K], bf16)
            r1 = const.tile([1, K], f32)
            r2 = const.tile([1, K], f32)
            ch = const.tile([1, K], bf16)
            cm = const.tile([1, K], bf16)
            cl = const.tile([1, K], bf16)
            nc.vector.tensor_copy(ch[:], c_row[:])
            nc.vector.tensor_sub(r1[:], c_row[:], ch[:])
            nc.vector.tensor_copy(cm[:], r1[:])
            nc.vector.tensor_sub(r2[:], r1[:], cm[:])
            nc.vector.tensor_copy(cl[:], r2[:])
            # compute engines cannot target base partitions 1/2 -> DMA instead
            nc.sync.dma_start(out=c3[0:1, :], in_=ch[:])
            nc.sync.dma_start(out=c3[1:2, :], in_=cm[:])
            nc.sync.dma_start(out=c3[2:3, :], in_=cl[:])
            ones3 = const.tile([3, P], bf16)
            nc.vector.memset(ones3[:], 1.0)

            # iota row 0..1023 on every partition (fp16: integers <= 2048 exact)
            iota = const.tile([P, K], fp16)
            nc.gpsimd.iota(iota[:], [[1, K]], channel_multiplier=0,
                           allow_small_or_imprecise_dtypes=True)

            idx_f = const.tile([P, nchunk], f32)
            idx_i = const.tile([P, nchunk], i32)

            # ---------------- main loop ----------------
            for g in range(ngroup):
                rows = slice(g * group * P, (g + 1) * group * P)
                xin = io_pool.tile([P, group, D], f32)
                nc.sync.dma_start(out=xin[:],
                                  in_=x[rows, :].rearrange("(c p) d -> p c d", p=P))
                for cc in range(group):
                    j = g * group + cc
                    # transpose x chunk -> [dim, tok]
                    xt_ps = psA.tile([P, P], f32, tag="t128")
                    nc.tensor.transpose(xt_ps[:], xin[:, cc, :], identity[:])
                    xt = work.tile([P, P], f32)
                    nc.scalar.copy(xt[:], xt_ps[:])

                    # v = x @ embT - esq/2 -> PSUM [tok, code]
                    # (main fp32 matmul, then a K=3 bf16 matmul accumulates
                    #  the per-code constant -||e_k||^2/2)
                    v = psV.tile([P, K], f32, tag="v")
                    for h in range(2):
                        sl = slice(h * 512, (h + 1) * 512)
                        if main_dtype == "float32r":
                            nc.tensor.matmul(v[:, sl],
                                             lhsT=xt[:].bitcast(f32r),
                                             rhs=embT[:, sl].bitcast(f32r),
                                             start=True, stop=False)
                        else:
                            nc.tensor.matmul(v[:, sl], lhsT=xt[:], rhs=embT[:, sl],
                                             start=True, stop=False)
                    for h in range(2):
                        sl = slice(h * 512, (h + 1) * 512)
                        nc.tensor.matmul(v[:, sl], lhsT=ones3[:], rhs=c3[:, sl],
                                         start=False, stop=True)

                    # maxv = row-max of v (DVE reads PSUM directly)
                    maxv = work.tile([P, 1], f32)
                    nc.vector.tensor_reduce(maxv[:], v[:],
                                            axis=mybir.AxisListType.X,
                                            op=mybir.AluOpType.max)

                    # index = sum over k of (v >= maxv) * k
                    junk = work.tile([P, K], fp16)
                    nc.vector.scalar_tensor_tensor(
                        out=junk[:], in0=v[:], scalar=maxv[:, 0:1],
                        in1=iota[:],
                        op0=mybir.AluOpType.is_ge, op1=mybir.AluOpType.mult,
                        accum_out=idx_f[:, j:j + 1])

                # int indices for this group (exact integers in fp32)
                gsl = slice(g * group, (g + 1) * group)
                nc.vector.tensor_copy(idx_i[:, gsl], idx_f[:, gsl])

                # gather embed rows per chunk: token p <- embed[idx[p]]
                for cc in range(group):
                    j = g * group + cc
                    q1 = work.tile([P, D], f32, tag="q1")
                    nc.gpsimd.indirect_dma_start(
                        out=q1[:], out_offset=None,
                        in_=embed[:],
                        in_offset=IndirectOffsetOnAxis(ap=idx_i[:, j:j + 1],
                                                       axis=0))
                    nc.sync.dma_start(out=quant[j * P:(j + 1) * P, :],
                                      in_=q1[:])

            nc.sync.dma_start(out=ind[:].rearrange("(c p) -> p c", p=P),
                              in_=idx_i[:])
    nc.compile()
    return nc


_NC_CACHE = {}

MAIN_DTYPE = os.environ.get("VQ_MAIN_DTYPE", "float32")


def _get_nc(tpc=TPC):
    key = (tpc, MAIN_DTYPE)
    if key not in _NC_CACHE:
        _NC_CACHE[key] = build(tpc=tpc, main_dtype=MAIN_DTYPE)
    return _NC_CACHE[key]


def kernel(x: np.ndarray, embed: np.ndarray, trace: bool = False):
    """Full-input entry point: shards x over 8 cores, runs, gathers."""
    shape = x.shape
    xf = np.ascontiguousarray(np.asarray(x, dtype=np.float32).reshape(-1, D))
    e = np.ascontiguousarray(np.asarray(embed, dtype=np.float32))
    n = xf.shape[0]
    assert n == NTOK, f"expected {NTOK} tokens, got {n}"
    tpc = n // NCORES

    nc = _get_nc(tpc)
    in_maps = [
        {"x": xf[i * tpc:(i + 1) * tpc], "embed": e} for i in range(NCORES)
    ]
    res = run_bass_kernel_spmd(nc, in_maps, core_ids=list(range(NCORES)),
                               trace=trace)
    quant = np.concatenate([r["quant"] for r in res.results], axis=0)
    indv = np.concatenate([r["ind"] for r in res.results], axis=0)
    out_q = quant.reshape(shape).astype(np.float32)
    out_i = indv.reshape(shape[:-1]).astype(np.int32)
    if trace:
        kernel.last_exec_time_ns = res.exec_time_ns
        kernel.last_mean_exec_time_ns = res.mean_exec_time_ns
    return out_q, out_i
